# revision 20
# baseline (speedup 1.0000x reference)
"""Distributed Trainium2 kernel for ANEMultiHeadAttention.

Problem: B=2, C=1024, S=2048, H=16, D=64.
  x: (B, C, 1, S);  q = Wq x + bq; k = Wk x; v = Wv x + bv
  per-head attention (softmax over keys), out = Wo o + bo.

Sharding (8 cores): core i handles batch b = i // 4 and head-group
hg = i % 4 (4 heads = 256 channels). Q/K/V column-parallel, Wo
row-parallel; host sums the 4 partial outputs per batch.
The v-bias contributes Wo @ bv (softmax rows sum to 1) and is folded
into a host-side constant along with bo.

Per-core device algorithm (all matmuls bf16, f32 PSUM):
  - q = WqT_s^T @ x (+bq), k = WkT_s^T @ x; vT = x^T @ WvT_s stored
    (128, head, 65) per key-tile with a ones column (the PV matmul then
    also accumulates softmax denominators).
  - attention in q-windows of 512, head pairs row-packed: BOTH heads'
    scoresT land in ONE (128, 1024) PSUM tile (different banks), one
    exp instruction (ACT, scale 1/8) covers both heads -> the exp
    stream on the Scalar engine runs back-to-back (it is the kernel's
    critical path); PV: o_aug += vT_aug^T @ expT per head.
  - QKV / vT / out-projection work is emission-interleaved into the
    attention loops so the PE stays dense while ACT works.
  - normalize (recip + partition_broadcast + mul) runs off the hot
    path after a fast PSUM->SBUF evac.
"""

import sys

for p in ("/opt/trn_rl_repo",):
    if p not in sys.path:
        sys.path.insert(0, p)

from contextlib import ExitStack

import ml_dtypes
import numpy as np

import concourse.bass as bass
import concourse.mybir as mybir
import concourse.tile as tile
from concourse import bacc
from concourse.bass_utils import run_bass_kernel_spmd

# Problem shape (hardcoded per contest rules)
B, C, S, H = 2, 1024, 2048, 16
D = C // H  # 64
N_CORES = 8
HG = 4  # head groups
HPG = H // HG  # heads per group = 4
CPG = HPG * D  # channels per group = 256
P = 128
NK = C // P  # 8 contraction tiles over C
NST = S // P  # 16 key tiles
QW = 512  # q window per head
NQW = S // QW  # 4
WIN = 1024  # qk-projection unit width

F32 = mybir.dt.float32
BF16 = mybir.dt.bfloat16
EXP = mybir.ActivationFunctionType.Exp

_CACHED_NC = None


def build_nc():
    nc = bacc.Bacc("TRN2", target_bir_lowering=False, debug=False)

    x_d = nc.dram_tensor("x", (P, NK, S), BF16, kind="ExternalInput")
    wq_d = nc.dram_tensor("wqT", (P, NK, CPG), BF16, kind="ExternalInput")
    wk_d = nc.dram_tensor("wkT", (P, NK, CPG), BF16, kind="ExternalInput")
    wv_d = nc.dram_tensor("wvT", (P, NK, CPG), BF16, kind="ExternalInput")
    wo_d = nc.dram_tensor("woT", (P, 2, C), BF16, kind="ExternalInput")
    bq_d = nc.dram_tensor("bq", (P, 2), F32, kind="ExternalInput")
    out_d = nc.dram_tensor("out", (P, NK, S), F32, kind="ExternalOutput")

    with tile.TileContext(nc) as tc, ExitStack() as ctx:
        const = ctx.enter_context(tc.tile_pool(name="const", bufs=1))
        work = ctx.enter_context(tc.tile_pool(name="work", bufs=1))
        expp = ctx.enter_context(tc.tile_pool(name="expp", bufs=8))
        onp = ctx.enter_context(tc.tile_pool(name="onp", bufs=6))
        outp = ctx.enter_context(tc.tile_pool(name="outp", bufs=4))
        smal = ctx.enter_context(tc.tile_pool(name="smal", bufs=6))
        # PSUM budget (8 banks): psc 2x[128,1024]f32 (4) + ppv oa/ob (2) +
        # sps shared small-unit pool (2)
        psc = ctx.enter_context(tc.tile_pool(name="psc", bufs=2, space="PSUM"))
        ppv = ctx.enter_context(tc.tile_pool(name="ppv", bufs=2, space="PSUM"))
        sps = ctx.enter_context(tc.tile_pool(name="sps", bufs=2, space="PSUM"))

        # ---- weights first (small, needed immediately), then x first-half,
        # then x second-half, so the pair-0 projections start earliest ----
        wq_sb = const.tile([P, NK, CPG], BF16, tag="wq")
        nc.sync.dma_start(wq_sb[:], wq_d[:])
        wk_sb = const.tile([P, NK, CPG], BF16, tag="wk")
        nc.sync.dma_start(wk_sb[:], wk_d[:])
        wv_sb = const.tile([P, NK, CPG], BF16, tag="wv")
        nc.sync.dma_start(wv_sb[:], wv_d[:])
        wo_sb = const.tile([P, 2, C], BF16, tag="wo")
        nc.sync.dma_start(wo_sb[:], wo_d[:])
        bq_sb = const.tile([P, 2], F32, tag="bq")
        nc.sync.dma_start(bq_sb[:], bq_d[:])
        xh = [[None, None] for _ in range(NK)]
        for half in range(2):
            for kt in range(NK):
                t = const.tile([P, WIN], BF16, tag=f"x{kt}h{half}", name=f"x{kt}h{half}")
                nc.sync.dma_start(
                    t[:], x_d[:, kt, half * WIN : (half + 1) * WIN]
                )
                xh[kt][half] = t

        # activations, one tile per (pair, window/chunk) for fine deps
        def wtile(nm):
            return work.tile([P, WIN], BF16, tag=nm, name=nm)

        k_t = [[wtile(f"k{p_}c{c}") for c in range(2)] for p_ in range(2)]
        q_t = [[wtile(f"q{p_}w{w}") for w in range(2)] for p_ in range(2)]
        o_t = [[wtile(f"o{p_}w{w}") for w in range(2)] for p_ in range(2)]

        vt = [
            work.tile([P, HPG, D + 1], BF16, tag=f"vt{st}", name=f"vt{st}")
            for st in range(NST)
        ]

        def qk_mms(ps, w_sb, pair, c, kt):
            for ch in range(2):
                nc.tensor.matmul(
                    ps[:, ch * 512 : (ch + 1) * 512],
                    w_sb[:, kt, pair * P : (pair + 1) * P],
                    xh[kt][c][:, ch * 512 : (ch + 1) * 512],
                    start=(kt == 0),
                    stop=(kt == NK - 1),
                )

        def qk_evac(ps, dst_ap, pair, bias):
            if bias:
                nc.vector.tensor_scalar_add(
                    dst_ap, ps[:], bq_sb[:, pair : pair + 1]
                )
            else:
                nc.vector.tensor_copy(dst_ap, ps[:])

        def qk_half(w_sb, dst, pair, c, ch, bias):
            """512-wide half of a q/k projection unit; 1-bank PSUM tile from
            the shared small-unit pool so score-tile ping-pong is untouched."""
            ps = sps.tile([P, QW], F32, tag="sps", name="ps_qkh")
            for kt in range(NK):
                nc.tensor.matmul(
                    ps[:],
                    w_sb[:, kt, pair * P : (pair + 1) * P],
                    xh[kt][c][:, ch * 512 : (ch + 1) * 512],
                    start=(kt == 0),
                    stop=(kt == NK - 1),
                )
            qk_evac(ps, dst[:, ch * 512 : (ch + 1) * 512], pair, bias)

        def vt_mm(ps, st, kt):
            half, off = divmod(st * P, WIN)
            nc.tensor.matmul(
                ps[:, :CPG],
                xh[kt][half][:, off : off + P],
                wv_sb[:, kt, :],
                start=(kt == 0),
                stop=(kt == NK - 1),
            )

        def vt_evac(ps, st):
            nc.vector.tensor_copy(
                vt[st][:, :, 0:D],
                ps[:, :CPG].rearrange("p (h d) -> p h d", h=HPG),
            )

        def vt_unit(st):
            nc.vector.memset(vt[st][:], 1.0)
            ps = sps.tile([P, QW], F32, tag="sps", name="ps_vt")
            for kt in range(NK):
                vt_mm(ps, st, kt)
            vt_evac(ps, st)

        def attention(pair, qw, inject=None):
            """Emits one q-window of attention; returns a `finish` closure
            (recip + broadcast + normalize-mul) for the caller to inject into
            a LATER call's kt loop, keeping the DVE FIFO clear at the call
            boundary (the PSUM->SBUF evac still runs here, eagerly)."""
            inject = inject or {}
            w, half = divmod(qw, 2)
            qs = slice(half * QW, (half + 1) * QW)
            oa = ppv.tile([P, QW], F32, tag="ppv", name="oa")
            ob = ppv.tile([P, QW], F32, tag="ppv", name="ob")

            def pv(prev):
                pkt, pe = prev
                for acc, hoff, cs in (
                    (oa, 0, slice(0, QW)),
                    (ob, 1, slice(QW, 2 * QW)),
                ):
                    nc.tensor.matmul(
                        acc[0 : D + 1, :],
                        vt[pkt][:, 2 * pair + hoff, :],
                        pe[:, cs],
                        start=(pkt == 0),
                        stop=(pkt == NST - 1),
                    )

            prev = None
            for kt in range(NST):
                s = psc.tile([P, WIN], F32, tag="psc", name="s")
                c, j = divmod(kt, NK)
                for rlo, rhi, cs, tpos in (
                    (0, D, slice(0, QW), (0, 0)),
                    (D, P, slice(QW, 2 * QW), (64, 0)),
                ):
                    nc.tensor.matmul(
                        s[:, cs],
                        k_t[pair][c][rlo:rhi, j * P : (j + 1) * P],
                        q_t[pair][w][rlo:rhi, qs],
                        tile_position=tpos,
                    )
                e = expp.tile([P, WIN], BF16, tag="exp", name="e")
                nc.scalar.activation(e[:], s[:], EXP, scale=float(D) ** -0.5)
                if prev is not None:
                    pv(prev)
                prev = (kt, e)
                for f in inject.get(kt, ()):
                    f()
            pv(prev)

            # fast evac to SBUF first (frees both PSUM slots); the recip/
            # broadcast/mul chain is returned as a closure so the caller can
            # defer it past the call boundary
            ous = []
            for acc in (oa, ob):
                ou = onp.tile([D + 1, QW], F32, tag="ou", name="ou")
                nc.vector.tensor_copy(ou[:], acc[0 : D + 1, :])
                ous.append(ou)

            def finish():
                for head, ou in enumerate(ous):
                    rc = smal.tile([1, QW], F32, tag="rc", name="rc")
                    nc.vector.reciprocal(rc[:], ou[D : D + 1, :])
                    rcb = smal.tile([D, QW], F32, tag="rcb", name="rcb")
                    nc.gpsimd.partition_broadcast(rcb[:], rc[:])
                    nc.vector.tensor_mul(
                        o_t[pair][w][head * D : (head + 1) * D, qs],
                        ou[0:D, :],
                        rcb[:],
                    )

            return finish

        def outproj_unit(qw, m):
            w, half = divmod(qw, 2)
            cs = slice(half * QW, (half + 1) * QW)
            ps = sps.tile([P, QW], F32, tag="sps", name="ps_out")
            for kt in range(2):
                nc.tensor.matmul(
                    ps[:],
                    wo_sb[:, kt, m * P : (m + 1) * P],
                    o_t[kt][w][:, cs],
                    start=(kt == 0),
                    stop=(kt == 1),
                )
            ot = outp.tile([P, QW], F32, tag="ot", name="ot")
            nc.vector.tensor_copy(ot[:], ps[:])
            nc.sync.dma_start(out_d[:, m, qw * QW : (qw + 1) * QW], ot[:])

        # ---- emission schedule ----
        # Head phase: k/q for pair 0 window 0, interleaved per contraction
        # tile so the (first-half) x DMAs pipeline straight into the PE.
        ps_k = psc.tile([P, WIN], F32, tag="psc", name="ps_k")
        ps_q = psc.tile([P, WIN], F32, tag="psc", name="ps_q")
        # HAM warm-up: ~4us of matmuls on the already-arrived weights while
        # the x DMAs land; results are discarded by the first start=True
        wk_flat = wk_sb[:].rearrange("p a b -> p (a b)")
        for i in range(10):
            nc.tensor.matmul(
                ps_k[:, 0:512] if i % 2 == 0 else ps_q[:, 0:512],
                wk_sb[:, 0, 0:P],
                wk_flat[:, 0:512],
            )
        for kt in range(NK):
            qk_mms(ps_k, wk_sb, 0, 0, kt)
            qk_mms(ps_q, wq_sb, 0, 0, kt)
        qk_evac(ps_k, k_t[0][0][:], 0, False)
        qk_evac(ps_q, q_t[0][0][:], 0, True)

        def U(f, *a):
            return lambda: f(*a)

        VT = lambda st: U(vt_unit, st)  # noqa: E731
        QKH = lambda wsb, dst, p_, c_, ch_, b_: U(  # noqa: E731
            qk_half, wsb, dst, p_, c_, ch_, b_
        )
        OP = lambda qw_, m_: U(outproj_unit, qw_, m_)  # noqa: E731

        f00 = attention(
            0,
            0,
            {
                0: (VT(0), VT(1)),
                1: (VT(2), VT(3)),
                3: (VT(4), VT(5)),
                5: (VT(6), VT(7)),
                7: (QKH(wk_sb, k_t[0][1], 0, 1, 0, False),),
                8: (VT(8), VT(9)),
                9: (QKH(wk_sb, k_t[0][1], 0, 1, 1, False),),
                10: (VT(10), VT(11)),
                12: (VT(12), VT(13)),
                14: (VT(14), VT(15)),
            },
        )
        f01 = attention(
            0,
            1,
            {
                2: (QKH(wk_sb, k_t[1][0], 1, 0, 0, False),),
                3: (QKH(wk_sb, k_t[1][0], 1, 0, 1, False),),
                4: (f00,),
                8: (QKH(wq_sb, q_t[1][0], 1, 0, 0, True),),
                9: (QKH(wq_sb, q_t[1][0], 1, 0, 1, True),),
                13: (QKH(wk_sb, k_t[1][1], 1, 1, 0, False),),
                14: (QKH(wk_sb, k_t[1][1], 1, 1, 1, False),),
            },
        )
        f10 = attention(1, 0, {3: (f01,)})
        f11 = attention(
            1,
            1,
            {
                2: (QKH(wq_sb, q_t[0][1], 0, 1, 0, True),),
                3: (QKH(wq_sb, q_t[0][1], 0, 1, 1, True),),
                4: (f10,),
                11: (OP(0, 0), OP(0, 1), OP(0, 2)),
                13: (OP(0, 3), OP(0, 4), OP(0, 5)),
                15: (OP(0, 6), OP(0, 7)),
            },
        )
        f02 = attention(
            0,
            2,
            {
                2: (QKH(wq_sb, q_t[1][1], 1, 1, 0, True),),
                3: (QKH(wq_sb, q_t[1][1], 1, 1, 1, True),),
                4: (f11,),
                11: (OP(1, 0), OP(1, 1), OP(1, 2)),
                13: (OP(1, 3), OP(1, 4), OP(1, 5)),
                15: (OP(1, 6), OP(1, 7)),
            },
        )
        f12 = attention(1, 2, {3: (f02,)})
        f03 = attention(
            0,
            3,
            {
                3: (f12,),
                11: (OP(2, 0), OP(2, 1), OP(2, 2)),
                13: (OP(2, 3), OP(2, 4), OP(2, 5)),
                15: (OP(2, 6), OP(2, 7)),
            },
        )
        f13 = attention(1, 3, {3: (f03,)})
        f13()
        for m in range(NK):
            outproj_unit(3, m)

    nc.compile()
    return nc


def _shard_inputs(hidden_states, Wq, bq, Wk, Wv, bv, Wo, bo):
    bf = ml_dtypes.bfloat16
    in_maps = []
    for core in range(N_CORES):
        b, hg = divmod(core, HG)
        x = hidden_states[b, :, 0, :]  # (C, S) f32
        cs = slice(hg * CPG, (hg + 1) * CPG)
        wqT = Wq[cs, :].T.reshape(NK, P, CPG).transpose(1, 0, 2)
        wkT = Wk[cs, :].T.reshape(NK, P, CPG).transpose(1, 0, 2)
        wvT = Wv[cs, :].T.reshape(NK, P, CPG).transpose(1, 0, 2)
        woT = Wo[:, cs].T.reshape(2, P, C).transpose(1, 0, 2)
        in_maps.append(
            {
                "x": np.ascontiguousarray(
                    x.reshape(NK, P, S).transpose(1, 0, 2)
                ).astype(bf),
                "wqT": np.ascontiguousarray(wqT).astype(bf),
                "wkT": np.ascontiguousarray(wkT).astype(bf),
                "wvT": np.ascontiguousarray(wvT).astype(bf),
                "woT": np.ascontiguousarray(woT).astype(bf),
                "bq": np.ascontiguousarray(
                    bq[cs].reshape(2, P).T
                ).astype(np.float32),
            }
        )
    return in_maps


def get_nc():
    global _CACHED_NC
    if _CACHED_NC is None:
        _CACHED_NC = build_nc()
    return _CACHED_NC


def run(hidden_states, Wq, bq, Wk, Wv, bv, Wo, bo, trace=False, **kw):
    nc = get_nc()
    in_maps = _shard_inputs(hidden_states, Wq, bq, Wk, Wv, bv, Wo, bo)
    res = run_bass_kernel_spmd(
        nc, in_maps, core_ids=list(range(N_CORES)), trace=trace, **kw
    )
    # unshard: sum partials per batch, add host-side constant bias
    bias_vec = (Wo.astype(np.float64) @ bv.astype(np.float64)).astype(
        np.float32
    ) + bo
    out = np.zeros((B, C, 1, S), dtype=np.float32)
    for core in range(N_CORES):
        b = core // HG
        part = np.asarray(res.results[core]["out"], dtype=np.float32)
        out[b, :, 0, :] += part.transpose(1, 0, 2).reshape(C, S)
    out[:, :, 0, :] += bias_vec[None, :, None]
    return out, res


def _run_subprocess(inputs):
    """Retry path for transient device failures: a fresh interpreter gets a
    fresh PJRT/device state."""
    import os
    import pickle
    import subprocess
    import tempfile

    kdir = os.path.dirname(os.path.abspath(__file__))
    with tempfile.TemporaryDirectory() as td:
        inp = os.path.join(td, "in.pkl")
        outp = os.path.join(td, "out.pkl")
        with open(inp, "wb") as f:
            pickle.dump(inputs, f)
        code = (
            "import pickle, sys; sys.path.insert(0, %r); import kernel;\n"
            "inputs = pickle.load(open(%r, 'rb'));\n"
            "out, _ = kernel.run(**inputs);\n"
            "pickle.dump(out, open(%r, 'wb'))\n" % (kdir, inp, outp)
        )
        subprocess.run([sys.executable, "-c", code], check=True, timeout=1500)
        with open(outp, "rb") as f:
            return pickle.load(f)


def kernel(**inputs):
    try:
        out, _ = run(**inputs)
        return out
    except Exception:
        pass
    # transient NRT_EXEC_UNIT_UNRECOVERABLE happens occasionally; retry in
    # fresh subprocesses (fresh device handles)
    last = None
    for _ in range(3):
        try:
            return _run_subprocess(inputs)
        except Exception as e:  # noqa: PERF203
            last = e
    raise last



# revision 25
# speedup vs baseline: 1.0699x; 1.0699x over previous
"""Distributed Trainium2 kernel for ANEMultiHeadAttention.

Problem: B=2, C=1024, S=2048, H=16, D=64.
  x: (B, C, 1, S);  q = Wq x + bq; k = Wk x; v = Wv x + bv
  per-head attention (softmax over keys), out = Wo o + bo.

Sharding (8 cores): core i handles batch b = i // 4 and head-group
hg = i % 4 (4 heads = 256 channels). Q/K/V column-parallel, Wo
row-parallel; host sums the 4 partial outputs per batch.
The v-bias contributes Wo @ bv (softmax rows sum to 1) and is folded
into a host-side constant along with bo.

Per-core device algorithm (all matmuls bf16, f32 PSUM):
  - q = WqT_s^T @ x (+bq), k = WkT_s^T @ x; vT = x^T @ WvT_s stored
    (128, head, 65) per key-tile with a ones column (the PV matmul then
    also accumulates softmax denominators).
  - attention in q-windows of 512, head pairs row-packed: BOTH heads'
    scoresT land in ONE (128, 1024) PSUM tile (different banks), one
    exp instruction (ACT, scale 1/8) covers both heads -> the exp
    stream on the Scalar engine runs back-to-back (it is the kernel's
    critical path); PV: o_aug += vT_aug^T @ expT per head.
  - QKV / vT / out-projection work is emission-interleaved into the
    attention loops so the PE stays dense while ACT works.
  - normalize (recip + partition_broadcast + mul) runs off the hot
    path after a fast PSUM->SBUF evac.
"""

import sys

for p in ("/opt/trn_rl_repo",):
    if p not in sys.path:
        sys.path.insert(0, p)

from contextlib import ExitStack

import ml_dtypes
import numpy as np

import concourse.bass as bass
import concourse.mybir as mybir
import concourse.tile as tile
from concourse import bacc
from concourse.bass_utils import run_bass_kernel_spmd

# Problem shape (hardcoded per contest rules)
B, C, S, H = 2, 1024, 2048, 16
D = C // H  # 64
N_CORES = 8
HG = 4  # head groups
HPG = H // HG  # heads per group = 4
CPG = HPG * D  # channels per group = 256
P = 128
NK = C // P  # 8 contraction tiles over C
NST = S // P  # 16 key tiles
QW = 512  # q window per head
NQW = S // QW  # 4
WIN = 1024  # qk-projection unit width

F32 = mybir.dt.float32
BF16 = mybir.dt.bfloat16
EXP = mybir.ActivationFunctionType.Exp

_CACHED_NC = None


def build_nc():
    nc = bacc.Bacc("TRN2", target_bir_lowering=False, debug=False)

    x_d = nc.dram_tensor("x", (P, NK, S), BF16, kind="ExternalInput")
    wq_d = nc.dram_tensor("wqT", (P, NK, CPG), BF16, kind="ExternalInput")
    wk_d = nc.dram_tensor("wkT", (P, NK, CPG), BF16, kind="ExternalInput")
    wv_d = nc.dram_tensor("wvT", (P, NK, CPG), BF16, kind="ExternalInput")
    wo_d = nc.dram_tensor("woT", (P, 2, C), BF16, kind="ExternalInput")
    bq_d = nc.dram_tensor("bq", (P, 2), F32, kind="ExternalInput")
    out_d = nc.dram_tensor("out", (P, NK, S), F32, kind="ExternalOutput")

    with tile.TileContext(nc) as tc, ExitStack() as ctx:
        const = ctx.enter_context(tc.tile_pool(name="const", bufs=1))
        work = ctx.enter_context(tc.tile_pool(name="work", bufs=1))
        expp = ctx.enter_context(tc.tile_pool(name="expp", bufs=8))
        onp = ctx.enter_context(tc.tile_pool(name="onp", bufs=6))
        outp = ctx.enter_context(tc.tile_pool(name="outp", bufs=4))
        smal = ctx.enter_context(tc.tile_pool(name="smal", bufs=6))
        # PSUM budget (8 banks): psc 2x[128,1024]f32 (4) + ppv oa/ob (2) +
        # sps shared small-unit pool (2)
        psc = ctx.enter_context(tc.tile_pool(name="psc", bufs=2, space="PSUM"))
        ppv = ctx.enter_context(tc.tile_pool(name="ppv", bufs=2, space="PSUM"))
        sps = ctx.enter_context(tc.tile_pool(name="sps", bufs=2, space="PSUM"))

        # ---- DMA order by first-use: bq (first q evac), wk+wq (head-phase
        # projections), x first-half, wv (vt units), x second-half, wo (out
        # proj, consumed ~70us in). Gets the first scores onto the PE at
        # ~11us instead of ~27us. ----
        bq_sb = const.tile([P, 2], F32, tag="bq")
        nc.sync.dma_start(bq_sb[:], bq_d[:])
        wk_sb = const.tile([P, NK, CPG], BF16, tag="wk")
        nc.sync.dma_start(wk_sb[:], wk_d[:])
        wq_sb = const.tile([P, NK, CPG], BF16, tag="wq")
        nc.sync.dma_start(wq_sb[:], wq_d[:])
        xh = [[None, None] for _ in range(NK)]
        for kt in range(NK):
            t = const.tile([P, WIN], BF16, tag=f"x{kt}h0", name=f"x{kt}h0")
            nc.sync.dma_start(t[:], x_d[:, kt, 0:WIN])
            xh[kt][0] = t
        wv_sb = const.tile([P, NK, CPG], BF16, tag="wv")
        nc.sync.dma_start(wv_sb[:], wv_d[:])
        for kt in range(NK):
            t = const.tile([P, WIN], BF16, tag=f"x{kt}h1", name=f"x{kt}h1")
            nc.sync.dma_start(t[:], x_d[:, kt, WIN : 2 * WIN])
            xh[kt][1] = t
        wo_sb = const.tile([P, 2, C], BF16, tag="wo")
        nc.sync.dma_start(wo_sb[:], wo_d[:])

        # activations, one tile per (pair, window/chunk) for fine deps
        def wtile(nm):
            return work.tile([P, WIN], BF16, tag=nm, name=nm)

        k_t = [[wtile(f"k{p_}c{c}") for c in range(2)] for p_ in range(2)]
        q_t = [[wtile(f"q{p_}w{w}") for w in range(2)] for p_ in range(2)]
        o_t = [[wtile(f"o{p_}w{w}") for w in range(2)] for p_ in range(2)]

        vt = [
            work.tile([P, HPG, D + 1], BF16, tag=f"vt{st}", name=f"vt{st}")
            for st in range(NST)
        ]

        def qk_mms(ps, w_sb, pair, c, kt):
            for ch in range(2):
                nc.tensor.matmul(
                    ps[:, ch * 512 : (ch + 1) * 512],
                    w_sb[:, kt, pair * P : (pair + 1) * P],
                    xh[kt][c][:, ch * 512 : (ch + 1) * 512],
                    start=(kt == 0),
                    stop=(kt == NK - 1),
                )

        def qk_evac(ps, dst_ap, pair, bias):
            if bias:
                nc.vector.tensor_scalar_add(
                    dst_ap, ps[:], bq_sb[:, pair : pair + 1]
                )
            else:
                nc.vector.tensor_copy(dst_ap, ps[:])

        def qk_half(w_sb, dst, pair, c, ch, bias):
            """512-wide half of a q/k projection unit; 1-bank PSUM tile from
            the shared small-unit pool so score-tile ping-pong is untouched."""
            ps = sps.tile([P, QW], F32, tag="sps", name="ps_qkh")
            for kt in range(NK):
                nc.tensor.matmul(
                    ps[:],
                    w_sb[:, kt, pair * P : (pair + 1) * P],
                    xh[kt][c][:, ch * 512 : (ch + 1) * 512],
                    start=(kt == 0),
                    stop=(kt == NK - 1),
                )
            qk_evac(ps, dst[:, ch * 512 : (ch + 1) * 512], pair, bias)

        def vt_mm(ps, st, kt):
            half, off = divmod(st * P, WIN)
            nc.tensor.matmul(
                ps[:, :CPG],
                xh[kt][half][:, off : off + P],
                wv_sb[:, kt, :],
                start=(kt == 0),
                stop=(kt == NK - 1),
            )

        def vt_evac(ps, st):
            nc.vector.tensor_copy(
                vt[st][:, :, 0:D],
                ps[:, :CPG].rearrange("p (h d) -> p h d", h=HPG),
            )

        def vt_unit(st):
            nc.vector.memset(vt[st][:], 1.0)
            ps = sps.tile([P, QW], F32, tag="sps", name="ps_vt")
            for kt in range(NK):
                vt_mm(ps, st, kt)
            vt_evac(ps, st)

        def attention(pair, qw, inject=None):
            """Emits one q-window of attention; returns a `finish` closure
            (recip + broadcast + normalize-mul) for the caller to inject into
            a LATER call's kt loop, keeping the DVE FIFO clear at the call
            boundary (the PSUM->SBUF evac still runs here, eagerly)."""
            inject = inject or {}
            w, half = divmod(qw, 2)
            qs = slice(half * QW, (half + 1) * QW)
            oa = ppv.tile([P, QW], F32, tag="ppv", name="oa")
            ob = ppv.tile([P, QW], F32, tag="ppv", name="ob")

            def pv(prev):
                pkt, pe = prev
                for acc, hoff, cs in (
                    (oa, 0, slice(0, QW)),
                    (ob, 1, slice(QW, 2 * QW)),
                ):
                    nc.tensor.matmul(
                        acc[0 : D + 1, :],
                        vt[pkt][:, 2 * pair + hoff, :],
                        pe[:, cs],
                        start=(pkt == 0),
                        stop=(pkt == NST - 1),
                    )

            prev = None
            for kt in range(NST):
                s = psc.tile([P, WIN], F32, tag="psc", name="s")
                c, j = divmod(kt, NK)
                for rlo, rhi, cs, tpos in (
                    (0, D, slice(0, QW), (0, 0)),
                    (D, P, slice(QW, 2 * QW), (64, 0)),
                ):
                    nc.tensor.matmul(
                        s[:, cs],
                        k_t[pair][c][rlo:rhi, j * P : (j + 1) * P],
                        q_t[pair][w][rlo:rhi, qs],
                        tile_position=tpos,
                    )
                e = expp.tile([P, WIN], BF16, tag="exp", name="e")
                nc.scalar.activation(e[:], s[:], EXP, scale=float(D) ** -0.5)
                if prev is not None:
                    pv(prev)
                prev = (kt, e)
                for f in inject.get(kt, ()):
                    f()
            pv(prev)

            # fast evac to SBUF first (frees both PSUM slots); the recip/
            # broadcast/mul chain is returned as a closure so the caller can
            # defer it past the call boundary
            ous = []
            for acc in (oa, ob):
                ou = onp.tile([D + 1, QW], F32, tag="ou", name="ou")
                nc.vector.tensor_copy(ou[:], acc[0 : D + 1, :])
                ous.append(ou)

            def finish():
                # both recips emitted before the muls: recip1 streams on the
                # DVE while gpsimd broadcasts rcb0, shaving ~1.4us of chain
                # latency vs interleaved emission
                rcbs = []
                for head, ou in enumerate(ous):
                    rc = smal.tile([1, QW], F32, tag="rc", name="rc")
                    nc.vector.reciprocal(rc[:], ou[D : D + 1, :])
                    rcb = smal.tile([D, QW], F32, tag="rcb", name="rcb")
                    nc.gpsimd.partition_broadcast(rcb[:], rc[:])
                    rcbs.append(rcb)
                for head, (ou, rcb) in enumerate(zip(ous, rcbs)):
                    nc.vector.tensor_mul(
                        o_t[pair][w][head * D : (head + 1) * D, qs],
                        ou[0:D, :],
                        rcb[:],
                    )

            return finish

        def outproj_unit(qw, m):
            w, half = divmod(qw, 2)
            cs = slice(half * QW, (half + 1) * QW)
            ps = sps.tile([P, QW], F32, tag="sps", name="ps_out")
            for kt in range(2):
                nc.tensor.matmul(
                    ps[:],
                    wo_sb[:, kt, m * P : (m + 1) * P],
                    o_t[kt][w][:, cs],
                    start=(kt == 0),
                    stop=(kt == 1),
                )
            ot = outp.tile([P, QW], F32, tag="ot", name="ot")
            nc.vector.tensor_copy(ot[:], ps[:])
            nc.sync.dma_start(out_d[:, m, qw * QW : (qw + 1) * QW], ot[:])

        # ---- emission schedule ----
        # Head phase: ONLY the first halves of k/q for pair 0 window 0 (the
        # minimum needed for c0's scores kt 0-3 and all its w=0 queries);
        # second halves are injected into c0. MMs interleave per contraction
        # tile so the x-half0 DMAs pipeline straight into the PE.
        ps_k = sps.tile([P, QW], F32, tag="sps", name="ps_k")
        ps_q = sps.tile([P, QW], F32, tag="sps", name="ps_q")
        # HAM warm-up: ~4us of matmuls on the already-arrived weights while
        # the x DMAs land; results are discarded by the first start=True
        wk_flat = wk_sb[:].rearrange("p a b -> p (a b)")
        for i in range(10):
            nc.tensor.matmul(
                ps_k[:] if i % 2 == 0 else ps_q[:],
                wk_sb[:, 0, 0:P],
                wk_flat[:, 0:512],
            )
        for kt in range(NK):
            for ps, w_sb in ((ps_k, wk_sb), (ps_q, wq_sb)):
                nc.tensor.matmul(
                    ps[:],
                    w_sb[:, kt, 0:P],
                    xh[kt][0][:, 0:512],
                    start=(kt == 0),
                    stop=(kt == NK - 1),
                )
        qk_evac(ps_k, k_t[0][0][:, 0:512], 0, False)
        qk_evac(ps_q, q_t[0][0][:, 0:512], 0, True)

        def U(f, *a):
            return lambda: f(*a)

        VT = lambda st: U(vt_unit, st)  # noqa: E731
        QKH = lambda wsb, dst, p_, c_, ch_, b_: U(  # noqa: E731
            qk_half, wsb, dst, p_, c_, ch_, b_
        )
        OP = lambda qw_, m_: U(outproj_unit, qw_, m_)  # noqa: E731

        # Injection load-balance: attention alone is ~15us PE per call vs
        # ~18.4us ACT, so each call can host ~3.5us of injected PE work
        # before the exp stream starves. c0 is structurally overloaded (all
        # 16 vt units are consumed by its own PV). finish closures are
        # ~8us of DVE chain; outproj units are placed in calls AFTER their
        # finish dependencies completed so their MMs never block the PE.
        f00 = attention(
            0,
            0,
            {
                0: (VT(0), VT(1), QKH(wk_sb, k_t[0][0], 0, 0, 1, False)),
                1: (VT(2), VT(3)),
                2: (QKH(wq_sb, q_t[0][0], 0, 0, 1, True),),
                3: (VT(4), VT(5)),
                5: (VT(6), VT(7), QKH(wk_sb, k_t[0][1], 0, 1, 0, False)),
                7: (QKH(wk_sb, k_t[0][1], 0, 1, 1, False),),
                8: (VT(8), VT(9)),
                10: (VT(10), VT(11)),
                12: (VT(12), VT(13)),
                14: (VT(14), VT(15)),
            },
        )
        f01 = attention(
            0,
            1,
            {
                0: (QKH(wk_sb, k_t[1][0], 1, 0, 0, False),),
                2: (QKH(wk_sb, k_t[1][0], 1, 0, 1, False),),
                4: (QKH(wq_sb, q_t[1][0], 1, 0, 0, True),),
                8: (f00,),
            },
        )
        f10 = attention(
            1,
            0,
            {
                0: (QKH(wk_sb, k_t[1][1], 1, 1, 0, False),),
                2: (QKH(wk_sb, k_t[1][1], 1, 1, 1, False),),
                4: (f01,),
                6: (QKH(wq_sb, q_t[1][0], 1, 0, 1, True),),
            },
        )
        f11 = attention(
            1,
            1,
            {
                0: (QKH(wq_sb, q_t[0][1], 0, 1, 0, True),),
                2: (QKH(wq_sb, q_t[0][1], 0, 1, 1, True),),
                4: (f10,),
            },
        )
        # OP(w, m) is placed only in calls where BOTH its finish deps
        # completed in an earlier call: an OP matmul emitted before its
        # o_t rows exist would block the strict-FIFO PE queue.
        f02 = attention(
            0,
            2,
            {
                0: (QKH(wq_sb, q_t[1][1], 1, 1, 0, True),),
                2: (QKH(wq_sb, q_t[1][1], 1, 1, 1, True),),
                4: (f11,),
                7: (OP(0, 0), OP(0, 1)),
                10: (OP(0, 2), OP(0, 3)),
            },
        )
        f12 = attention(
            1,
            2,
            {
                0: (OP(0, 4), OP(0, 5)),
                2: (OP(0, 6), OP(0, 7)),
                4: (f02,),
                7: (OP(1, 0), OP(1, 1)),
                10: (OP(1, 2), OP(1, 3)),
            },
        )
        f03 = attention(
            0,
            3,
            {
                0: (OP(1, 4), OP(1, 5)),
                2: (OP(1, 6), OP(1, 7)),
                4: (f12,),
            },
        )
        f13 = attention(
            1,
            3,
            {
                0: (OP(2, 0), OP(2, 1)),
                2: (OP(2, 2), OP(2, 3)),
                4: (f03,),
                7: (OP(2, 4), OP(2, 5)),
                10: (OP(2, 6), OP(2, 7)),
            },
        )
        f13()
        for m in range(NK):
            outproj_unit(3, m)

    nc.compile()
    return nc


def _shard_inputs(hidden_states, Wq, bq, Wk, Wv, bv, Wo, bo):
    bf = ml_dtypes.bfloat16
    in_maps = []
    for core in range(N_CORES):
        b, hg = divmod(core, HG)
        x = hidden_states[b, :, 0, :]  # (C, S) f32
        cs = slice(hg * CPG, (hg + 1) * CPG)
        wqT = Wq[cs, :].T.reshape(NK, P, CPG).transpose(1, 0, 2)
        wkT = Wk[cs, :].T.reshape(NK, P, CPG).transpose(1, 0, 2)
        wvT = Wv[cs, :].T.reshape(NK, P, CPG).transpose(1, 0, 2)
        woT = Wo[:, cs].T.reshape(2, P, C).transpose(1, 0, 2)
        in_maps.append(
            {
                "x": np.ascontiguousarray(
                    x.reshape(NK, P, S).transpose(1, 0, 2)
                ).astype(bf),
                "wqT": np.ascontiguousarray(wqT).astype(bf),
                "wkT": np.ascontiguousarray(wkT).astype(bf),
                "wvT": np.ascontiguousarray(wvT).astype(bf),
                "woT": np.ascontiguousarray(woT).astype(bf),
                "bq": np.ascontiguousarray(
                    bq[cs].reshape(2, P).T
                ).astype(np.float32),
            }
        )
    return in_maps


def get_nc():
    global _CACHED_NC
    if _CACHED_NC is None:
        _CACHED_NC = build_nc()
    return _CACHED_NC


def run(hidden_states, Wq, bq, Wk, Wv, bv, Wo, bo, trace=False, **kw):
    nc = get_nc()
    in_maps = _shard_inputs(hidden_states, Wq, bq, Wk, Wv, bv, Wo, bo)
    res = run_bass_kernel_spmd(
        nc, in_maps, core_ids=list(range(N_CORES)), trace=trace, **kw
    )
    # unshard: sum partials per batch, add host-side constant bias
    bias_vec = (Wo.astype(np.float64) @ bv.astype(np.float64)).astype(
        np.float32
    ) + bo
    out = np.zeros((B, C, 1, S), dtype=np.float32)
    for core in range(N_CORES):
        b = core // HG
        part = np.asarray(res.results[core]["out"], dtype=np.float32)
        out[b, :, 0, :] += part.transpose(1, 0, 2).reshape(C, S)
    out[:, :, 0, :] += bias_vec[None, :, None]
    return out, res


def _run_subprocess(inputs):
    """Retry path for transient device failures: a fresh interpreter gets a
    fresh PJRT/device state."""
    import os
    import pickle
    import subprocess
    import tempfile

    kdir = os.path.dirname(os.path.abspath(__file__))
    with tempfile.TemporaryDirectory() as td:
        inp = os.path.join(td, "in.pkl")
        outp = os.path.join(td, "out.pkl")
        with open(inp, "wb") as f:
            pickle.dump(inputs, f)
        code = (
            "import pickle, sys; sys.path.insert(0, %r); import kernel;\n"
            "inputs = pickle.load(open(%r, 'rb'));\n"
            "out, _ = kernel.run(**inputs);\n"
            "pickle.dump(out, open(%r, 'wb'))\n" % (kdir, inp, outp)
        )
        subprocess.run([sys.executable, "-c", code], check=True, timeout=1500)
        with open(outp, "rb") as f:
            return pickle.load(f)


def kernel(**inputs):
    try:
        out, _ = run(**inputs)
        return out
    except Exception:
        pass
    # transient NRT_EXEC_UNIT_UNRECOVERABLE happens occasionally; retry in
    # fresh subprocesses (fresh device handles)
    last = None
    for _ in range(3):
        try:
            return _run_subprocess(inputs)
        except Exception as e:  # noqa: PERF203
            last = e
    raise last



# revision 33
# speedup vs baseline: 1.0733x; 1.0032x over previous
"""Distributed Trainium2 kernel for ANEMultiHeadAttention.

Problem: B=2, C=1024, S=2048, H=16, D=64.
  x: (B, C, 1, S);  q = Wq x + bq; k = Wk x; v = Wv x + bv
  per-head attention (softmax over keys), out = Wo o + bo.

Sharding (8 cores): core i handles batch b = i // 4 and head-group
hg = i % 4 (4 heads = 256 channels). Q/K/V column-parallel, Wo
row-parallel; host sums the 4 partial outputs per batch.
The v-bias contributes Wo @ bv (softmax rows sum to 1) and is folded
into a host-side constant along with bo.

Per-core device algorithm (all matmuls bf16, f32 PSUM):
  - q = WqT_s^T @ x (+bq), k = WkT_s^T @ x; vT = x^T @ WvT_s stored
    (128, head, 65) per key-tile with a ones column (the PV matmul then
    also accumulates softmax denominators).
  - attention in q-windows of 512, head pairs row-packed: BOTH heads'
    scoresT land in ONE (128, 1024) PSUM tile (different banks), one
    exp instruction (ACT, scale 1/8) covers both heads -> the exp
    stream on the Scalar engine runs back-to-back (it is the kernel's
    critical path); PV: o_aug += vT_aug^T @ expT per head.
  - QKV / vT / out-projection work is emission-interleaved into the
    attention loops so the PE stays dense while ACT works.
  - normalize (recip + partition_broadcast + mul) runs off the hot
    path after a fast PSUM->SBUF evac.
"""

import sys

for p in ("/opt/trn_rl_repo",):
    if p not in sys.path:
        sys.path.insert(0, p)

from contextlib import ExitStack

import ml_dtypes
import numpy as np

import concourse.bass as bass
import concourse.mybir as mybir
import concourse.tile as tile
from concourse import bacc
from concourse.bass_utils import run_bass_kernel_spmd

# Problem shape (hardcoded per contest rules)
B, C, S, H = 2, 1024, 2048, 16
D = C // H  # 64
N_CORES = 8
HG = 4  # head groups
HPG = H // HG  # heads per group = 4
CPG = HPG * D  # channels per group = 256
P = 128
NK = C // P  # 8 contraction tiles over C
NST = S // P  # 16 key tiles
QW = 512  # q window per head
NQW = S // QW  # 4
WIN = 1024  # qk-projection unit width

F32 = mybir.dt.float32
BF16 = mybir.dt.bfloat16
EXP = mybir.ActivationFunctionType.Exp

_CACHED_NC = None


def build_nc():
    nc = bacc.Bacc("TRN2", target_bir_lowering=False, debug=False)

    x_d = nc.dram_tensor("x", (P, NK, S), BF16, kind="ExternalInput")
    wkq_d = nc.dram_tensor("wkqT", (P, 2, NK, CPG), BF16, kind="ExternalInput")
    wv_d = nc.dram_tensor("wvT", (P, NK, CPG), BF16, kind="ExternalInput")
    wo_d = nc.dram_tensor("woT", (P, 2, C), BF16, kind="ExternalInput")
    bq_d = nc.dram_tensor("bq", (P, 2), F32, kind="ExternalInput")
    out_d = nc.dram_tensor("out", (P, NK, S), F32, kind="ExternalOutput")

    with tile.TileContext(nc) as tc, ExitStack() as ctx:
        const = ctx.enter_context(tc.tile_pool(name="const", bufs=1))
        work = ctx.enter_context(tc.tile_pool(name="work", bufs=1))
        expp = ctx.enter_context(tc.tile_pool(name="expp", bufs=8))
        onp = ctx.enter_context(tc.tile_pool(name="onp", bufs=6))
        outp = ctx.enter_context(tc.tile_pool(name="outp", bufs=4))
        smal = ctx.enter_context(tc.tile_pool(name="smal", bufs=6))
        # PSUM budget (8 banks): psc 2x[128,1024]f32 (4) + ppv oa/ob (2) +
        # sps shared small-unit pool (2)
        psc = ctx.enter_context(tc.tile_pool(name="psc", bufs=2, space="PSUM"))
        ppv = ctx.enter_context(tc.tile_pool(name="ppv", bufs=2, space="PSUM"))
        sps = ctx.enter_context(tc.tile_pool(name="sps", bufs=2, space="PSUM"))

        # ---- DMA order by first-use: bq + a tiny warm-up slice first (the
        # warm-up matmuls start ~8us and warm the HAM clock), packed wk+wq
        # (head-phase projections), x first-half, wv (vt units), x
        # second-half, wo (out proj, consumed ~70us in). ----
        bq_sb = const.tile([P, 2], F32, tag="bq")
        nc.sync.dma_start(bq_sb[:], bq_d[:])
        xw_sb = const.tile([P, QW], BF16, tag="xw")
        nc.sync.dma_start(xw_sb[:], x_d[:, 0, 0:QW])
        wkq_sb = const.tile([P, 2, NK, CPG], BF16, tag="wkq")
        nc.sync.dma_start(wkq_sb[:], wkq_d[:])
        wk_sb = wkq_sb[:, 0]
        wq_sb = wkq_sb[:, 1]
        xh = [[None, None] for _ in range(NK)]
        for kt in range(NK):
            t = const.tile([P, WIN], BF16, tag=f"x{kt}h0", name=f"x{kt}h0")
            nc.sync.dma_start(t[:], x_d[:, kt, 0:WIN])
            xh[kt][0] = t
        wv_sb = const.tile([P, NK, CPG], BF16, tag="wv")
        nc.sync.dma_start(wv_sb[:], wv_d[:])
        for kt in range(NK):
            t = const.tile([P, WIN], BF16, tag=f"x{kt}h1", name=f"x{kt}h1")
            nc.sync.dma_start(t[:], x_d[:, kt, WIN : 2 * WIN])
            xh[kt][1] = t
        wo_sb = const.tile([P, 2, C], BF16, tag="wo")
        nc.sync.dma_start(wo_sb[:], wo_d[:])

        # activations, one tile per (pair, window/chunk) for fine deps
        def wtile(nm):
            return work.tile([P, WIN], BF16, tag=nm, name=nm)

        k_t = [[wtile(f"k{p_}c{c}") for c in range(2)] for p_ in range(2)]
        q_t = [[wtile(f"q{p_}w{w}") for w in range(2)] for p_ in range(2)]
        o_t = [[wtile(f"o{p_}w{w}") for w in range(2)] for p_ in range(2)]

        vt = [
            work.tile([P, HPG, D + 1], BF16, tag=f"vt{st}", name=f"vt{st}")
            for st in range(NST)
        ]

        def qk_mms(ps, w_sb, pair, c, kt):
            for ch in range(2):
                nc.tensor.matmul(
                    ps[:, ch * 512 : (ch + 1) * 512],
                    w_sb[:, kt, pair * P : (pair + 1) * P],
                    xh[kt][c][:, ch * 512 : (ch + 1) * 512],
                    start=(kt == 0),
                    stop=(kt == NK - 1),
                )

        def qk_evac(ps, dst_ap, pair, bias):
            if bias:
                nc.vector.tensor_scalar_add(
                    dst_ap, ps[:], bq_sb[:, pair : pair + 1]
                )
            else:
                nc.vector.tensor_copy(dst_ap, ps[:])

        def qk_half(w_sb, dst, pair, c, ch, bias):
            """512-wide half of a q/k projection unit; 1-bank PSUM tile from
            the shared small-unit pool so score-tile ping-pong is untouched."""
            ps = sps.tile([P, QW], F32, tag="sps", name="ps_qkh")
            for kt in range(NK):
                nc.tensor.matmul(
                    ps[:],
                    w_sb[:, kt, pair * P : (pair + 1) * P],
                    xh[kt][c][:, ch * 512 : (ch + 1) * 512],
                    start=(kt == 0),
                    stop=(kt == NK - 1),
                )
            qk_evac(ps, dst[:, ch * 512 : (ch + 1) * 512], pair, bias)

        def vt_mm(ps, st, kt):
            half, off = divmod(st * P, WIN)
            nc.tensor.matmul(
                ps[:, :CPG],
                xh[kt][half][:, off : off + P],
                wv_sb[:, kt, :],
                start=(kt == 0),
                stop=(kt == NK - 1),
            )

        def vt_evac(ps, st):
            nc.vector.tensor_copy(
                vt[st][:, :, 0:D],
                ps[:, :CPG].rearrange("p (h d) -> p h d", h=HPG),
            )

        def vt_unit(st):
            nc.vector.memset(vt[st][:], 1.0)
            ps = sps.tile([P, QW], F32, tag="sps", name="ps_vt")
            for kt in range(NK):
                vt_mm(ps, st, kt)
            vt_evac(ps, st)

        def attention(pair, qw, inject=None):
            """Emits one q-window of attention; returns a `finish` closure
            (recip + broadcast + normalize-mul) for the caller to inject into
            a LATER call's kt loop, keeping the DVE FIFO clear at the call
            boundary (the PSUM->SBUF evac still runs here, eagerly)."""
            inject = inject or {}
            w, half = divmod(qw, 2)
            qs = slice(half * QW, (half + 1) * QW)
            oa = ppv.tile([P, QW], F32, tag="ppv", name="oa")
            ob = ppv.tile([P, QW], F32, tag="ppv", name="ob")

            def pv(prev):
                pkt, pe = prev
                for acc, hoff, cs in (
                    (oa, 0, slice(0, QW)),
                    (ob, 1, slice(QW, 2 * QW)),
                ):
                    nc.tensor.matmul(
                        acc[0 : D + 1, :],
                        vt[pkt][:, 2 * pair + hoff, :],
                        pe[:, cs],
                        start=(pkt == 0),
                        stop=(pkt == NST - 1),
                    )

            prev = None
            for kt in range(NST):
                s = psc.tile([P, WIN], F32, tag="psc", name="s")
                c, j = divmod(kt, NK)
                for rlo, rhi, cs, tpos in (
                    (0, D, slice(0, QW), (0, 0)),
                    (D, P, slice(QW, 2 * QW), (64, 0)),
                ):
                    nc.tensor.matmul(
                        s[:, cs],
                        k_t[pair][c][rlo:rhi, j * P : (j + 1) * P],
                        q_t[pair][w][rlo:rhi, qs],
                        tile_position=tpos,
                    )
                e = expp.tile([P, WIN], BF16, tag="exp", name="e")
                nc.scalar.activation(e[:], s[:], EXP, scale=float(D) ** -0.5)
                if prev is not None:
                    pv(prev)
                prev = (kt, e)
                for f in inject.get(kt, ()):
                    f()
            pv(prev)

            # fast evac to SBUF first (frees both PSUM slots); bf16 from here
            # on - 0.4% denominator error is well inside the accuracy budget
            # and 16-bit DVE ops run at 2x rate
            ous = []
            with nc.allow_low_precision(reason="bf16 softmax-denominator path"):
                for acc in (oa, ob):
                    ou = onp.tile([D + 1, QW], BF16, tag="ou", name="ou")
                    nc.vector.tensor_copy(ou[:], acc[0 : D + 1, :])
                    ous.append(ou)

            def finish():
                # both recips emitted before the muls: recip1 streams on the
                # DVE while gpsimd broadcasts rcb0, shaving ~1.4us of chain
                # latency vs interleaved emission
                with nc.allow_low_precision(
                    reason="bf16 softmax-denominator path"
                ):
                    rcbs = []
                    for head, ou in enumerate(ous):
                        rc = smal.tile([1, QW], BF16, tag="rc", name="rc")
                        nc.vector.reciprocal(rc[:], ou[D : D + 1, :])
                        rcb = smal.tile([D, QW], BF16, tag="rcb", name="rcb")
                        nc.gpsimd.partition_broadcast(rcb[:], rc[:])
                        rcbs.append(rcb)
                    for head, (ou, rcb) in enumerate(zip(ous, rcbs)):
                        nc.vector.tensor_mul(
                            o_t[pair][w][head * D : (head + 1) * D, qs],
                            ou[0:D, :],
                            rcb[:],
                        )

            return finish

        def outproj_unit(qw, m):
            w, half = divmod(qw, 2)
            cs = slice(half * QW, (half + 1) * QW)
            ps = sps.tile([P, QW], F32, tag="sps", name="ps_out")
            for kt in range(2):
                nc.tensor.matmul(
                    ps[:],
                    wo_sb[:, kt, m * P : (m + 1) * P],
                    o_t[kt][w][:, cs],
                    start=(kt == 0),
                    stop=(kt == 1),
                )
            ot = outp.tile([P, QW], F32, tag="ot", name="ot")
            nc.vector.tensor_copy(ot[:], ps[:])
            nc.sync.dma_start(out_d[:, m, qw * QW : (qw + 1) * QW], ot[:])

        # ---- emission schedule ----
        # Head phase: ONLY the first halves of k/q for pair 0 window 0 (the
        # minimum needed for c0's scores kt 0-3 and all its w=0 queries);
        # second halves are injected into c0. MMs interleave per contraction
        # tile so the x-half0 DMAs pipeline straight into the PE.
        ps_k = sps.tile([P, QW], F32, tag="sps", name="ps_k")
        ps_q = sps.tile([P, QW], F32, tag="sps", name="ps_q")
        # HAM warm-up: ~4us of matmuls on the tiny first-DMA'd x slice while
        # the weight DMAs land; results are discarded by the first start=True
        for i in range(10):
            nc.tensor.matmul(
                ps_k[:] if i % 2 == 0 else ps_q[:],
                xw_sb[:, 0:P],
                xw_sb[:],
            )
        for kt in range(NK):
            for ps, w_sb in ((ps_k, wk_sb), (ps_q, wq_sb)):
                nc.tensor.matmul(
                    ps[:],
                    w_sb[:, kt, 0:P],
                    xh[kt][0][:, 0:512],
                    start=(kt == 0),
                    stop=(kt == NK - 1),
                )
        qk_evac(ps_k, k_t[0][0][:, 0:512], 0, False)
        qk_evac(ps_q, q_t[0][0][:, 0:512], 0, True)

        def U(f, *a):
            return lambda: f(*a)

        VT = lambda st: U(vt_unit, st)  # noqa: E731
        QKH = lambda wsb, dst, p_, c_, ch_, b_: U(  # noqa: E731
            qk_half, wsb, dst, p_, c_, ch_, b_
        )
        OP = lambda qw_, m_: U(outproj_unit, qw_, m_)  # noqa: E731

        # Injection load-balance: attention alone is ~15us PE per call vs
        # ~18.4us ACT, so each call can host ~3.5us of injected PE work
        # before the exp stream starves. c0 is structurally overloaded (all
        # 16 vt units are consumed by its own PV). finish closures are
        # ~8us of DVE chain; outproj units are placed in calls AFTER their
        # finish dependencies completed so their MMs never block the PE.
        f00 = attention(
            0,
            0,
            {
                0: (VT(0), VT(1), QKH(wk_sb, k_t[0][0], 0, 0, 1, False)),
                1: (VT(2), VT(3)),
                2: (QKH(wq_sb, q_t[0][0], 0, 0, 1, True),),
                3: (VT(4), VT(5)),
                5: (VT(6), VT(7), QKH(wk_sb, k_t[0][1], 0, 1, 0, False)),
                7: (QKH(wk_sb, k_t[0][1], 0, 1, 1, False),),
                8: (VT(8), VT(9)),
                10: (VT(10), VT(11)),
                12: (VT(12), VT(13)),
                14: (VT(14), VT(15)),
            },
        )
        f01 = attention(
            0,
            1,
            {
                0: (QKH(wk_sb, k_t[1][0], 1, 0, 0, False),),
                2: (QKH(wk_sb, k_t[1][0], 1, 0, 1, False),),
                4: (QKH(wq_sb, q_t[1][0], 1, 0, 0, True),),
                8: (f00,),
            },
        )
        f10 = attention(
            1,
            0,
            {
                0: (QKH(wk_sb, k_t[1][1], 1, 1, 0, False),),
                2: (QKH(wk_sb, k_t[1][1], 1, 1, 1, False),),
                4: (f01,),
                6: (QKH(wq_sb, q_t[1][0], 1, 0, 1, True),),
            },
        )
        f11 = attention(
            1,
            1,
            {
                0: (QKH(wq_sb, q_t[0][1], 0, 1, 0, True),),
                2: (QKH(wq_sb, q_t[0][1], 0, 1, 1, True),),
                4: (f10,),
            },
        )
        # OP(w, m) placement rules: (a) only in calls where BOTH finish deps
        # completed in an earlier call - an OP matmul emitted before its o_t
        # rows exist blocks the strict-FIFO PE queue; (b) one unit per 2-kt
        # slot, never paired - the 2-slot sps pool frees via the DVE evac
        # copy, and a unit emitted in the same slot as its predecessor
        # stalls the PE on the slot-reuse semaphore.
        f02 = attention(
            0,
            2,
            {
                0: (QKH(wq_sb, q_t[1][1], 1, 1, 0, True),),
                2: (QKH(wq_sb, q_t[1][1], 1, 1, 1, True),),
                4: (f11,),
                6: (OP(0, 0),),
                8: (OP(0, 1),),
                10: (OP(0, 2),),
                12: (OP(0, 3),),
                14: (OP(0, 4),),
            },
        )
        f12 = attention(
            1,
            2,
            {
                0: (OP(0, 5),),
                2: (OP(0, 6),),
                4: (f02, OP(0, 7)),
                6: (OP(1, 0),),
                8: (OP(1, 1),),
                10: (OP(1, 2),),
                12: (OP(1, 3),),
                14: (OP(1, 4),),
            },
        )
        f03 = attention(
            0,
            3,
            {
                0: (OP(1, 5),),
                2: (OP(1, 6),),
                4: (f12, OP(1, 7)),
            },
        )
        f13 = attention(
            1,
            3,
            {
                0: (OP(2, 0),),
                2: (OP(2, 1),),
                4: (f03, OP(2, 2)),
                6: (OP(2, 3),),
                8: (OP(2, 4),),
                10: (OP(2, 5),),
                12: (OP(2, 6),),
                14: (OP(2, 7),),
            },
        )
        f13()
        for m in range(NK):
            outproj_unit(3, m)

    nc.compile()
    return nc


def _shard_inputs(hidden_states, Wq, bq, Wk, Wv, bv, Wo, bo):
    bf = ml_dtypes.bfloat16
    in_maps = []
    for core in range(N_CORES):
        b, hg = divmod(core, HG)
        x = hidden_states[b, :, 0, :]  # (C, S) f32
        cs = slice(hg * CPG, (hg + 1) * CPG)
        wqT = Wq[cs, :].T.reshape(NK, P, CPG).transpose(1, 0, 2)
        wkT = Wk[cs, :].T.reshape(NK, P, CPG).transpose(1, 0, 2)
        wvT = Wv[cs, :].T.reshape(NK, P, CPG).transpose(1, 0, 2)
        woT = Wo[:, cs].T.reshape(2, P, C).transpose(1, 0, 2)
        wkqT = np.stack([wkT, wqT], axis=1)  # (P, 2, NK, CPG)
        in_maps.append(
            {
                "x": np.ascontiguousarray(
                    x.reshape(NK, P, S).transpose(1, 0, 2)
                ).astype(bf),
                "wkqT": np.ascontiguousarray(wkqT).astype(bf),
                "wvT": np.ascontiguousarray(wvT).astype(bf),
                "woT": np.ascontiguousarray(woT).astype(bf),
                "bq": np.ascontiguousarray(
                    bq[cs].reshape(2, P).T
                ).astype(np.float32),
            }
        )
    return in_maps


def get_nc():
    global _CACHED_NC
    if _CACHED_NC is None:
        _CACHED_NC = build_nc()
    return _CACHED_NC


def run(hidden_states, Wq, bq, Wk, Wv, bv, Wo, bo, trace=False, **kw):
    nc = get_nc()
    in_maps = _shard_inputs(hidden_states, Wq, bq, Wk, Wv, bv, Wo, bo)
    res = run_bass_kernel_spmd(
        nc, in_maps, core_ids=list(range(N_CORES)), trace=trace, **kw
    )
    # unshard: sum partials per batch, add host-side constant bias
    bias_vec = (Wo.astype(np.float64) @ bv.astype(np.float64)).astype(
        np.float32
    ) + bo
    out = np.zeros((B, C, 1, S), dtype=np.float32)
    for core in range(N_CORES):
        b = core // HG
        part = np.asarray(res.results[core]["out"], dtype=np.float32)
        out[b, :, 0, :] += part.transpose(1, 0, 2).reshape(C, S)
    out[:, :, 0, :] += bias_vec[None, :, None]
    return out, res


def _run_subprocess(inputs):
    """Retry path for transient device failures: a fresh interpreter gets a
    fresh PJRT/device state."""
    import os
    import pickle
    import subprocess
    import tempfile

    kdir = os.path.dirname(os.path.abspath(__file__))
    with tempfile.TemporaryDirectory() as td:
        inp = os.path.join(td, "in.pkl")
        outp = os.path.join(td, "out.pkl")
        with open(inp, "wb") as f:
            pickle.dump(inputs, f)
        code = (
            "import pickle, sys; sys.path.insert(0, %r); import kernel;\n"
            "inputs = pickle.load(open(%r, 'rb'));\n"
            "out, _ = kernel.run(**inputs);\n"
            "pickle.dump(out, open(%r, 'wb'))\n" % (kdir, inp, outp)
        )
        subprocess.run([sys.executable, "-c", code], check=True, timeout=1500)
        with open(outp, "rb") as f:
            return pickle.load(f)


def kernel(**inputs):
    try:
        out, _ = run(**inputs)
        return out
    except Exception:
        pass
    # transient NRT_EXEC_UNIT_UNRECOVERABLE happens occasionally; retry in
    # fresh subprocesses (fresh device handles)
    last = None
    for _ in range(3):
        try:
            return _run_subprocess(inputs)
        except Exception as e:  # noqa: PERF203
            last = e
    raise last

